# revision 1
# baseline (speedup 1.0000x reference)
"""GAT layer (gnn_message_passing) Trainium2 Bass kernel, v3.

Data-parallel over batch B=8, one graph per NeuronCore.  HW exec ~52us
(baseline 169us).

Host-side LAYOUT transforms (no model math): node_mask kills ~50% of
nodes, and masked rows/columns of the attention matrix contribute
nothing, so the host ships the compacted kept-node subset (J = JB*128
padded): xk [J,D] f32, xkT [D,J] bf16 (pre-transposed), additive
adjacency mask adjm[j,i] = 0 (edge) / -1e4 bf16, a packed weight blob
[WT | a_l|a_r | W], and an identity matrix (building it on GPSIMD would
stall behind the Pool ucode library load).  Kept rows are scattered
back into the full [N,D] output on the host.

Device math, per core, on the compacted graph:
  h  = xk @ W;  el = xk @ (W a_l);  er = xk @ (W a_r)   (PE)
  e  = lrelu(el_i + er_j + adjm_ji)    additive mask: lrelu(-1e4+x)
                                       stays huge-negative -> exp = 0
  pm = exp(e)  -> fp8e4                (ScalarE, one table: Prelu+Exp+Ln
                                        all live in natural_log_exp set)
  oT = h^T pm; rs = 1^T pm             (PE fp8 DoubleRow, 2 j-blocks per
                                        pass; rowsum needs >=16 weight
                                        cols so 16 dup rows, read row 0)
  out = LN(oT^T / rs + xk)             (r folded via ACT scale= AP)

Scheduling notes (engine queues are in-order, so emission order is
placement): h/el/er copies are emitted inside the main j-loop right
before their consumers; lrelu runs on ScalarE (Prelu) for
ACT_LRELU_BLOCKS and as max(p, 0.2p) on DVE otherwise (tensor_scalar
q=0.2p, not an erq column, so nothing waits on the whole h pipeline);
el is partition-broadcast with a ones (x) el_row PE matmul;
tensor_scalar with TWO vector-scalar operands hits a ~2us slow path on
HW, so the LN scale/shift uses single-scalar ops (ACT fuses
z*rstd - mu*rstd via bias=/scale= APs).
"""

import os
import sys

import numpy as np

if "/opt/trn_rl_repo" not in sys.path:
    sys.path.insert(0, "/opt/trn_rl_repo")

B, N, D = 8, 2048, 128
ALPHA = 0.2
EPS = 1e-5
NEG = -10000.0
NCORES = 8

_PROG_CACHE = {}
RACE_DETECT = True
SEM_CLEAR_MODE = "skip"  # runtime resets sems between executions (verified)
LAST_EXEC_TIME_NS = None
LAST_MEAN_EXEC_TIME_NS = None


def _knob(name, default):
    v = os.environ.get(name)
    if v is None or v == "":
        return frozenset(default)
    if v == "-":
        return frozenset()
    return frozenset(int(x) for x in v.split(","))


def _patch_sem_clear():
    """This environment's walrus rejects EVENT_SEMAPHORE_RANGE_CLEAR
    ("ISA wrong length").  Tail sem reset is unnecessary here (runtime
    restores sems between executions), so skip it."""
    import bass_rust
    import concourse.bass as bass

    if getattr(bass.BassEngine, "_gat_sem_clear_patched", False):
        return

    def sem_clear(self, sem):
        if SEM_CLEAR_MODE == "skip":
            return None
        if not isinstance(sem, range):
            sem = range(sem.num, sem.num + 1)
        net = {s: 0 for s in sem}
        for b in self.bass.m.functions[0].blocks:
            for inst in b.instructions:
                si = inst.sync_info
                if si is None or not si.on_update:
                    continue
                for u in si.on_update:
                    if u.id in net:
                        if u.update_mode in ("sem-add-imm", "sem-inc"):
                            net[u.id] += u.update_value if u.update_value is not None else 1
                        elif u.update_mode in ("sem-dec",):
                            net[u.id] -= u.update_value if u.update_value is not None else 1
                        else:
                            raise AssertionError(u.update_mode)
        last = None
        for s in sem:
            if net[s]:
                h = bass_rust.SemaphoreHandle(name=f"semdec_{s}", num=s)
                last = self.sem_inc(h, -net[s])
        return last

    bass.BassEngine.sem_clear = sem_clear
    bass.BassEngine._gat_sem_clear_patched = True


def _split_waits(nc, mybir, max_waits=1):
    """This walrus build allows only one semaphore-wait slot per
    instruction; hoist extra waits onto standalone EventSemaphore
    carriers immediately before the offender on the same engine."""
    for f in nc.m.functions:
        for b in f.blocks:
            il = b.instructions
            k = 0
            while k < len(il):
                i = il[k]
                si = i.sync_info
                if si is not None and si.on_wait and len(si.on_wait) > max_waits:
                    waits = list(si.on_wait)
                    extra, keep = waits[:-max_waits], waits[-max_waits:]
                    for j, w in enumerate(extra):
                        ev = mybir.InstEventSemaphore(
                            name=f"{i.name}-wsplit{j}",
                            engine=i.engine,
                            debug=i.debug,
                            sync_info=mybir.SyncInfo(on_wait=[w], on_update=[]),
                        )
                        il.insert(k + j, ev)
                    k += len(extra)
                    i.sync_info = mybir.SyncInfo(
                        on_wait=keep, on_update=list(si.on_update or []))
                k += 1
    return nc


def _build_program(jb_count: int, apply_affine: bool):
    import concourse.bass as bass
    import concourse.tile as tile
    from concourse import mybir
    from concourse.masks import make_identity

    _patch_sem_clear()

    JB = jb_count
    J = JB * 128
    # which j-blocks do lrelu on the Scalar engine (Prelu) vs DVE, and which
    # DVE-route blocks push the +adjm mask-add onto GPSIMD
    act_lrelu = _knob("GAT_ACT_LRELU", range(JB)[2::4])
    pool_uadd = _knob("GAT_POOL_UADD", [])
    use_fp8 = os.environ.get("GAT_FP8", "1") != "0"

    fp32 = mybir.dt.float32
    bf16 = mybir.dt.bfloat16
    f8 = mybir.dt.float8e4
    h_dt = f8 if use_fp8 else bf16
    A = mybir.AluOpType
    F = mybir.ActivationFunctionType
    DR = mybir.MatmulPerfMode.DoubleRow

    nc = bass.Bass(use_seq_codegen=True, detect_race_conditions=RACE_DETECT)

    xk_in = nc.declare_dram_parameter("xk", [J, D], fp32, isOutput=False)
    adjm = nc.declare_dram_parameter("adjm", [J, J], bf16, isOutput=False)
    wblob_in = nc.declare_dram_parameter("wblob", [D, 2 * D + 2], bf16,
                                         isOutput=False)
    xkt_in = nc.declare_dram_parameter("xkt", [D, J], bf16, isOutput=False)
    id_in = nc.declare_dram_parameter("ident", [128, 128], fp32, isOutput=False)
    if apply_affine:
        g_in = nc.declare_dram_parameter("gamma", [D], fp32, isOutput=False)
        b_in = nc.declare_dram_parameter("beta", [D], fp32, isOutput=False)
    out_d = nc.declare_dram_parameter("out", [J, D], bf16, isOutput=True)

    el_dram = nc.dram_tensor("el_scratch", [J], bf16)

    # PSUM-bank-aligned i-chunks for matmul outputs
    chunks = []
    s = 0
    while s < J:
        chunks.append((s, min(512, J - s)))
        s += 512

    def bcast(ap, parts=128):
        return bass.AP(tensor=ap.tensor, offset=ap.offset, ap=[[0, parts]] + list(ap.ap))

    with tile.TileContext(nc) as tc:
        with tc.tile_pool(name="persist", bufs=1) as per:
            # identity arrives by DMA: building it on GPSIMD would stall
            # everything behind the Pool ucode library load
            ident_f32 = per.tile([128, 128], fp32)
            # DoubleRow LDWEIGHTS requires >=16 weight columns; all 16 output
            # partitions then hold the same rowsum and we read partition 0
            ones_col = per.tile([128, 2, 16], h_dt)
            nc.vector.memset(ones_col, 1.0)
            ones_row = per.tile([1, 128], bf16)
            nc.vector.memset(ones_row, 1.0)
            eps_col = per.tile([128, 1], fp32)
            nc.vector.memset(eps_col, EPS)
            ident_bf = per.tile([128, 128], bf16)

            xk_all = per.tile([128, JB, D], fp32)
            adj_all = per.tile([128, JB, J], bf16)
            xkT_all = per.tile([128, JB, D], bf16)
            h_all = per.tile([128, JB, D], h_dt)
            elr_col = per.tile([128, JB, 2], fp32)   # [:, :, 0]=el, [:, :, 1]=er
            el_row = per.tile([1, J], bf16)
            el_bc = per.tile([128, J], bf16)
            oT_sb = per.tile([128, J], bf16)
            z_all = per.tile([128, JB, D], fp32)
            o_all = per.tile([128, JB, D], bf16)
            mv_all = per.tile([128, JB, 2], fp32)
            r_col = per.tile([128, JB], fp32)
            rstd = per.tile([128, JB], fp32)

            # Input DMAs, critical-path-ordered on the single sync HWDGE
            # queue (each dma_start costs ~0.6us of serial descriptor gen):
            # weights+xkT feed el (which gates the main loop), adj groups
            # next, then inputs only the epilogue needs (ident, xk).
            w_sb = per.tile([128, 2 * D + 4], bf16)  # [WT | al|ar | W | wl|wr]
            nc.sync.dma_start(
                out=xkT_all,
                in_=xkt_in[:, :].rearrange("p (b d) -> p b d", d=128))
            nc.sync.dma_start(out=w_sb[:, :2 * D + 2],
                              in_=wblob_in[:, :])
            agrp = (JB + 2) // 3
            for g0 in range(0, JB, agrp):
                gn = min(agrp, JB - g0)
                nc.sync.dma_start(
                    out=adj_all[:, g0:g0 + gn, :],
                    in_=adjm[g0 * 128:(g0 + gn) * 128, :].rearrange(
                        "(b p) i -> p b i", p=128))
            nc.sync.dma_start(out=ident_f32, in_=id_in[:, :])
            nc.vector.tensor_copy(out=ident_bf, in_=ident_f32)
            nc.sync.dma_start(
                out=xk_all, in_=xk_in[:, :].rearrange("(b p) d -> p b d", p=128))
            if apply_affine:
                g_bc = per.tile([128, D], fp32)
                nc.sync.dma_start(out=g_bc, in_=bcast(g_in[:]))
                b_bc = per.tile([128, D], fp32)
                nc.sync.dma_start(out=b_bc, in_=bcast(b_in[:]))

            # PE p-state warmup: harmless matmuls so the el chain below
            # runs at full clock instead of the 0.65GHz cold state
            with tc.tile_pool(name="wu_ps", bufs=1, space="PSUM") as wup:
                wu_ps = wup.tile([128, 128], fp32, tag="wu")
                for _ in range(8):
                    nc.tensor.matmul(wu_ps, lhsT=ones_row, rhs=ones_row,
                                     start=True, stop=True)

            # ---- preprocessing: wlr, el (gates main loop), then h -------
            # w_sb layout: [WT(0:D) | alr(D:D+2) | W(D+2:2D+2) | wlr(...)]
            W_OFF = D + 2
            half = (J // 2) // 128 * 128
            with (
                tc.tile_pool(name="pp_ps1", bufs=1, space="PSUM") as pp_ps1,
            ):
                wlr_ps = pp_ps1.tile([128, 2], fp32, tag="wlr")
                nc.tensor.matmul(wlr_ps, lhsT=w_sb[:, 0:D],
                                 rhs=w_sb[:, D:D + 2], start=True, stop=True)
                nc.vector.tensor_copy(out=w_sb[:, 2 * D + 2:2 * D + 4],
                                      in_=wlr_ps)

                el_ps = pp_ps1.tile([1, J], fp32, tag="el")
                xkT_flat = xkT_all[:].rearrange("p b d -> p (b d)")
                for cs, cn in chunks:
                    # el row chunk: el = wl^T @ xkT
                    nc.tensor.matmul(el_ps[:, cs:cs + cn],
                                     lhsT=w_sb[:, 2 * D + 2:2 * D + 3],
                                     rhs=xkT_flat[:, cs:cs + cn],
                                     start=True, stop=True)
                # el row -> SBUF bf16 halves
                nc.scalar.copy(out=el_row[:, :half], in_=el_ps[:, :half])
                nc.vector.tensor_copy(out=el_row[:, half:], in_=el_ps[:, half:])

            # partition-broadcast el via PE: ones[1,128] (x) el_row[1,J]
            with tc.tile_pool(name="bc_ps", bufs=1, space="PSUM") as bcp:
                bc_ps = bcp.tile([128, J], fp32, tag="bc")
                for cs, cn in chunks:
                    nc.tensor.matmul(bc_ps[:, cs:cs + cn],
                                     lhsT=ones_row,
                                     rhs=el_row[:, cs:cs + cn],
                                     start=True, stop=True)
                nc.scalar.copy(out=el_bc[:, :half], in_=bc_ps[:, :half])
                nc.vector.tensor_copy(out=el_bc[:, half:], in_=bc_ps[:, half:])

            # ---- main loop over j-blocks --------------------------------
            with (
                tc.tile_pool(name="mm_ps", bufs=1, space="PSUM") as mm_ps_pool,
                tc.tile_pool(name="rs_ps", bufs=1, space="PSUM") as rs_ps_pool,
                tc.tile_pool(name="ublk", bufs=6) as ublk,
            ):
                oT_ps = mm_ps_pool.tile([128, J], fp32)
                rs_ps = rs_ps_pool.tile([16, J], fp32)

                # j-block pairs run DoubleRow fp8 matmuls (2 k-tiles per
                # pass); an odd tail block falls back to a plain matmul
                npairs = JB // 2 if use_fp8 else 0
                ngroups = npairs + (JB - 2 * npairs)
                gwidth = 2 if use_fp8 else 1

                def emit_mms(g):
                    st, sp = (g == 0), (g == ngroups - 1)
                    rhs = pexp_pairs[g]
                    if g < npairs:
                        lhs_o = h_all[:, 2 * g:2 * g + 2, :]
                        lhs_r = ones_col
                        pm = DR
                    else:
                        blk = 2 * npairs + (g - npairs)
                        lhs_o = h_all[:, blk, :]
                        lhs_r = ones_col[:, 0, :]
                        pm = None
                    mm_groups = [(oT_ps, lhs_o), (rs_ps, lhs_r)]
                    if sp:
                        mm_groups.reverse()
                    for out_ps, lhs in mm_groups:
                        for cs, cn in chunks:
                            r = (rhs[:, :, cs:cs + cn] if g < npairs
                                 else rhs[:, 0, cs:cs + cn])
                            nc.tensor.matmul(out_ps[:, cs:cs + cn],
                                             lhsT=lhs, rhs=r,
                                             start=st, stop=sp,
                                             perf_mode=pm,
                                             skip_group_check=True)

                pexp_pairs = {}
                pp_ps_cm = tc.tile_pool(name="pp_ps", bufs=2, space="PSUM")
                pp_ps = pp_ps_cm.__enter__()
                for jb in range(JB):
                    # h / el / er for this block (emitted here so the copies
                    # sit in each engine queue right before this block's use)
                    he_ps = pp_ps.tile([128, D + 2], fp32, tag="he")
                    nc.tensor.matmul(he_ps, lhsT=xkT_all[:, jb, :],
                                     rhs=w_sb[:, W_OFF:W_OFF + D + 2],
                                     start=True, stop=True)
                    if jb % 2 == 0:
                        nc.scalar.copy(out=h_all[:, jb, :], in_=he_ps[:, :D])
                    else:
                        nc.vector.tensor_copy(out=h_all[:, jb, :],
                                              in_=he_ps[:, :D])
                    nc.vector.tensor_copy(out=elr_col[:, jb, :],
                                          in_=he_ps[:, D:D + 2])

                    g, gh = jb // gwidth, jb % gwidth
                    if g not in pexp_pairs:
                        pexp_pairs[g] = ublk.tile([128, gwidth, J], h_dt,
                                                  name=f"pexp{g}", tag="pexp")
                    adj_t = adj_all[:, jb, :]
                    er_s = elr_col[:, jb, 1:2]
                    u = ublk.tile([128, J], bf16, tag="u")
                    if jb in act_lrelu:
                        w_t = ublk.tile([128, J], bf16, tag="w")
                        nc.vector.tensor_tensor(out=w_t, in0=adj_t, in1=el_bc,
                                                op=A.add)
                        nc.scalar.activation(out=u, in_=w_t, func=F.Prelu,
                                             bias=er_s, scale=1.0, alpha=ALPHA)
                    else:
                        p = ublk.tile([128, J], bf16, tag="p")
                        nc.vector.tensor_scalar(
                            out=p, in0=el_bc, scalar1=er_s, scalar2=None,
                            op0=A.add)
                        q = ublk.tile([128, J], bf16, tag="q")
                        nc.vector.tensor_scalar(
                            out=q, in0=p, scalar1=ALPHA, scalar2=None,
                            op0=A.mult)
                        u0 = ublk.tile([128, J], bf16, tag="u0")
                        nc.vector.tensor_tensor(out=u0, in0=p, in1=q, op=A.max)
                        eng = nc.gpsimd if jb in pool_uadd else nc.vector
                        eng.tensor_tensor(out=u, in0=u0, in1=adj_t, op=A.add)
                    nc.scalar.activation(out=pexp_pairs[g][:, gh, :], in_=u,
                                         func=F.Exp)
                    if gh == gwidth - 1 or jb == JB - 1:
                        emit_mms(g)
                pp_ps_cm.__exit__(None, None, None)

                # rowsum first (its accumulation finished before oT in the
                # last group): row [1,J] -> col [128,JB] via DMA bounce +
                # PE transpose, then reciprocal
                rs_sb = ublk.tile([1, J], fp32, tag="rs_sb")
                half2 = (J // 2) // 128 * 128
                nc.scalar.copy(out=rs_sb[:, :half2], in_=rs_ps[0:1, :half2])
                nc.vector.tensor_copy(out=rs_sb[:, half2:], in_=rs_ps[0:1, half2:])

                # rowsum row -> column via JB tiny PE transposes (no DMA)
                with tc.tile_pool(name="rs2_ps", bufs=1, space="PSUM") as rs2:
                    rsc_ps = rs2.tile([128, JB], fp32, tag="rsc")
                    for ib in range(JB):
                        nc.tensor.transpose(
                            rsc_ps[:, ib:ib + 1],
                            rs_sb[:, ib * 128:(ib + 1) * 128],
                            ident_f32[:1, :1])
                    nc.vector.reciprocal(out=r_col, in_=rsc_ps)

                # oT PSUM -> SBUF in two halves on ACT + DVE
                nc.scalar.copy(out=oT_sb[:, :half2], in_=oT_ps[:, :half2])
                nc.vector.tensor_copy(out=oT_sb[:, half2:], in_=oT_ps[:, half2:])

            # ---- epilogue: normalize, residual, layernorm ---------------
            with (
                tc.tile_pool(name="ep", bufs=6) as ep,
                tc.tile_pool(name="ep_ps", bufs=3, space="PSUM") as ep_ps,
            ):
                for ib in range(JB):
                    tr_ps = ep_ps.tile([128, 128], bf16, tag="tr")
                    nc.tensor.transpose(tr_ps, oT_sb[:, ib * 128:(ib + 1) * 128],
                                        ident_bf)
                    z1 = ep.tile([128, 128], fp32, tag="z1")
                    if ib % 2 == 0:
                        nc.scalar.activation(out=z1, in_=tr_ps, func=F.Identity,
                                             bias=0.0,
                                             scale=r_col[:, ib:ib + 1])
                        zeng = nc.vector
                    else:
                        nc.vector.tensor_scalar(
                            out=z1, in0=tr_ps, scalar1=r_col[:, ib:ib + 1],
                            scalar2=None, op0=A.mult)
                        zeng = nc.gpsimd
                    zeng.tensor_tensor(out=z_all[:, ib, :], in0=z1,
                                       in1=xk_all[:, ib, :], op=A.add)
                    st6 = ep.tile([128, 6], fp32, tag="st6")
                    nc.vector.bn_stats(out=st6, in_=z_all[:, ib, :])
                    nc.vector.bn_aggr(out=mv_all[:, ib, :], in_=st6)

                # rstd = exp(-0.5*ln(var+eps)), batched (ln/exp table)
                var_v = mv_all[:, :, 1:2].rearrange("p b o -> p (b o)")
                lnv = ep.tile([128, JB], fp32, tag="lnv")
                nc.scalar.activation(out=lnv, in_=var_v, func=F.Ln,
                                     bias=eps_col, scale=1.0)
                nc.scalar.activation(out=rstd, in_=lnv, func=F.Exp, scale=-0.5)

                # -mu*rstd for the fused ACT pass (z*rstd + (-mu*rstd))
                mr = ep.tile([128, JB], fp32, tag="mr")
                nc.vector.tensor_tensor(out=mr, in0=mv_all[:, :, 0], in1=rstd,
                                        op=A.mult)
                nmr = ep.tile([128, JB], fp32, tag="nmr")
                nc.vector.tensor_scalar(out=nmr, in0=mr, scalar1=-1.0,
                                        scalar2=None, op0=A.mult)

                for ib in range(JB):
                    o_t = o_all[:, ib, :]
                    # tensor_scalar with TWO vector scalars hits a ~2us slow
                    # path on HW; use single-scalar ops instead
                    if ib % 2 == 1:
                        nc.scalar.activation(
                            out=o_t, in_=z_all[:, ib, :], func=F.Identity,
                            bias=nmr[:, ib:ib + 1], scale=rstd[:, ib:ib + 1])
                    else:
                        tz = ep.tile([128, 128], fp32, tag="tz")
                        nc.vector.tensor_scalar(
                            out=tz, in0=z_all[:, ib, :],
                            scalar1=mv_all[:, ib, 0:1], scalar2=None,
                            op0=A.subtract)
                        nc.vector.tensor_scalar(
                            out=o_t, in0=tz, scalar1=rstd[:, ib:ib + 1],
                            scalar2=None, op0=A.mult)
                    if apply_affine:
                        nc.vector.tensor_tensor(out=o_t, in0=o_t, in1=g_bc,
                                                op=A.mult)
                        nc.vector.tensor_tensor(out=o_t, in0=o_t, in1=b_bc,
                                                op=A.add)
                    if ib % 3 == 2 or ib == JB - 1:
                        lo = (ib // 3) * 3
                        nc.sync.dma_start(
                            out=out_d[lo * 128:(ib + 1) * 128, :].rearrange(
                                "(b p) d -> p b d", p=128),
                            in_=o_all[:, lo:ib + 1, :])
    return _split_waits(nc, mybir)


def _get_program(jb_count: int, apply_affine: bool):
    key = (jb_count, apply_affine, os.environ.get("GAT_ACT_LRELU"))
    if key not in _PROG_CACHE:
        _PROG_CACHE[key] = _build_program(jb_count, apply_affine)
    return _PROG_CACHE[key]


def _prep_inputs(x, adj_bool, node_mask, W, a_l, a_r, gamma, beta,
                 apply_affine, keeps, J):
    import ml_dtypes

    bf16 = ml_dtypes.bfloat16
    x = np.asarray(x, dtype=np.float32)
    adj_bool = np.asarray(adj_bool)
    w_np = np.asarray(W, dtype=np.float32)
    alr_np = np.stack([np.asarray(a_l, np.float32),
                       np.asarray(a_r, np.float32)], axis=1)
    # w_sb layout expected on device: [WT | al|ar | W]
    wblob = np.ascontiguousarray(
        np.concatenate([w_np.T, alr_np, w_np], axis=1).astype(bf16))
    ident = np.eye(128, dtype=np.float32)
    in_maps = []
    for b in range(NCORES):
        keep = keeps[b]
        K = len(keep)
        xk = np.zeros((J, D), dtype=np.float32)
        xk[:K] = x[b][keep]
        # adjm[j, i] = 0 if edge(keep_i <- keep_j) else -1e4
        adjm = np.full((J, J), NEG, dtype=np.float32)
        sub = adj_bool[b][np.ix_(keep, keep)]          # [i, j]
        adjm[:K, :K] = (sub.T.astype(np.float32) - 1.0) * (-NEG)
        m = {
            "xk": xk,
            "xkt": np.ascontiguousarray(xk.T.astype(bf16)),
            "ident": ident,
            "adjm": np.ascontiguousarray(adjm.astype(bf16)),
            "wblob": wblob,
        }
        if apply_affine:
            m["gamma"] = np.ascontiguousarray(np.asarray(gamma, np.float32))
            m["beta"] = np.ascontiguousarray(np.asarray(beta, np.float32))
        in_maps.append(m)
    return in_maps


def kernel(x, adj_bool, node_mask, W, a_l, a_r, gamma, beta):
    global LAST_EXEC_TIME_NS, LAST_MEAN_EXEC_TIME_NS
    from concourse.bass_utils import run_bass_kernel_spmd

    gamma_np = np.asarray(gamma, dtype=np.float32)
    beta_np = np.asarray(beta, dtype=np.float32)
    apply_affine = not (np.all(gamma_np == 1.0) and np.all(beta_np == 0.0))

    node_mask = np.asarray(node_mask)
    keeps = [np.flatnonzero(node_mask[b]) for b in range(NCORES)]
    kmax = max(max(len(k) for k in keeps), 1)
    JB = (kmax + 127) // 128
    J = JB * 128

    nc = _get_program(JB, apply_affine)
    in_maps = _prep_inputs(x, adj_bool, node_mask, W, a_l, a_r,
                           gamma_np, beta_np, apply_affine, keeps, J)
    trace = bool(int(os.environ.get("GAT_TRACE", "0")))
    res = run_bass_kernel_spmd(nc, in_maps, list(range(NCORES)), trace=trace)
    LAST_EXEC_TIME_NS = res.exec_time_ns
    LAST_MEAN_EXEC_TIME_NS = res.mean_exec_time_ns

    out = np.zeros((NCORES, N, D), dtype=np.float32)
    if apply_affine:
        out[:] = beta_np[None, None, :]
    for b in range(NCORES):
        keep = keeps[b]
        dev = np.asarray(res.results[b]["out"], dtype=np.float32)
        out[b][keep] = dev[:len(keep)]
    return out

